# revision 4
# baseline (speedup 1.0000x reference)
"""MiniGRU Trainium2 kernel.

Problem: h_t = (1-z_t) h_{t-1} + z_t g(p_t), with
  z_t = sigmoid(x_t @ Wz^T + bz), p_t = x_t @ Wh^T + bh,
  g(x) = x + 0.5 for x>=0 else sigmoid(x)  (note g(x) = max(x+0.5, sigmoid(x))),
  initial state g(h_0).  Shapes: x [4, 4096, 1024], H = 1024.

Sharding: 8 cores = batch(4) x H-halves(2). No collectives. Each core gets
host-pre-transposed inputs:
  xT  [1024 din, 4096 seq]   (moving operand for both GEMMs)
  wzT/whT [1024 din, 512 ch] (stationary operands)
  aux [128, 5, 4]            per chan-group columns: g(h0), bz, -bz, bh, bh+0.5
and returns hT [512 ch, 4096 seq]; host transposes back.

Device dataflow per (seq-block of nb, chan-group of 128):
  PE: accumulating matmuls -> PSUM kz, kh  [128 ch, nb seq]
  ACT: a = sigmoid(-kz-bz), z = sigmoid(kz+bz), sp = sigmoid(kh+bh)
  DVE: gp = max(kh+(bh+0.5), sp); b = z*gp;
       h = tensor_tensor_scan(a, b, init)  -- state = a*state + b along seq
Scan state chains across seq-blocks via initial=prev_h[:, -1:].

The GEMM inputs go to the PE as float16 (x and W quantized on host): fp16
streams at the same 1 column/cycle as fp32r but its stationary loads get the
automatic Fast Weight Load path (2 elems per 32-bit read), halving the
per-matmul LDWEIGHTS bubble that dominates the fp32r schedule. PSUM
accumulation stays fp32, as does everything downstream of the GEMMs.
"""

import numpy as np

import concourse.bass as bass
import concourse.bacc as bacc
import concourse.mybir as mybir
import concourse.tile as tile
from concourse.bass_utils import run_bass_kernel_spmd

F32 = mybir.dt.float32
F32R = mybir.dt.float32r
F16 = mybir.dt.float16
BF16 = mybir.dt.bfloat16
AF = mybir.ActivationFunctionType
ALU = mybir.AluOpType

BS, SEQ, DIN, H = 4, 4096, 1024, 1024
NCORES = 8
H_SPLIT = 2
CH = H // H_SPLIT  # channels per core

IN_DT = {"f32r": F32R, "f16": F16, "bf16": BF16}


def build_nc(seq=SEQ, din=DIN, ch=CH, nb=512, x_bufs=4, loop_reps=1,
             epool_bufs=3, h_bufs=4, psum_bufs=None, dt_in="f32r", korder="j"):
    """Build the single-core SPMD Bass program.

    loop_reps > 1 wraps the whole body in a hardware For_i loop that
    recomputes the same output N times — used only for benchmarking
    (slope of wall time vs reps isolates HW exec time from RPC overhead).

    korder="k" orders the accumulation loops stationary-major (all PSUM
    row-chunks for one weight tile before moving to the next), reusing the
    loaded stationary across nb//512 matmuls.
    """
    kt = din // 128   # contraction tiles
    mg = ch // 128    # chan groups
    nblk = seq // nb  # seq blocks
    if psum_bufs is None:
        psum_bufs = max(1, 8 // (2 * (nb // 512)))  # kz+kh tags fill all 8 banks

    dt = IN_DT[dt_in]
    nc = bacc.Bacc("TRN2", target_bir_lowering=False, debug=False)

    xT_d = nc.dram_tensor("xT", [din, seq], dt, kind="ExternalInput")
    wzT_d = nc.dram_tensor("wzT", [din, ch], dt, kind="ExternalInput")
    whT_d = nc.dram_tensor("whT", [din, ch], dt, kind="ExternalInput")
    aux_d = nc.dram_tensor("aux", [128, 5, mg], F32, kind="ExternalInput")
    hT_d = nc.dram_tensor("hT", [ch, seq], F32, kind="ExternalOutput")

    xT_r = xT_d.ap().rearrange("(k p) s -> p k s", p=128)
    wzT_r = wzT_d.ap().rearrange("(k p) c -> p k c", p=128)
    whT_r = whT_d.ap().rearrange("(k p) c -> p k c", p=128)

    with tile.TileContext(nc) as tc:
        with (
            tc.tile_pool(name="wpool", bufs=1) as wpool,
            tc.tile_pool(name="xpool", bufs=x_bufs) as xpool,
            tc.tile_pool(name="epool", bufs=epool_bufs) as epool,
            tc.tile_pool(name="hpool", bufs=1) as hpool,
            tc.tile_pool(name="psum", bufs=psum_bufs, space="PSUM") as psum,
        ):
            wz_sb = wpool.tile([128, kt, ch], dt)
            wh_sb = wpool.tile([128, kt, ch], dt)
            aux_sb = wpool.tile([128, 5, mg], F32)
            nc.sync.dma_start(aux_sb[:], aux_d.ap())
            # per-k W loads so the first matmul waits only on its own slice;
            # issued on the scalar HWDGE ring so they don't queue ahead of the
            # first x-block loads on the sync ring.
            for k in range(kt):
                nc.scalar.dma_start(wz_sb[:, k, :], wzT_r[:, k, :])
                nc.scalar.dma_start(wh_sb[:, k, :], whT_r[:, k, :])

            def emit_mms(psum_t, w_sb, xb, ms, nmm):
                if korder == "j":
                    for j in range(nmm):
                        js = slice(j * 512, (j + 1) * 512)
                        for k in range(kt):
                            nc.tensor.matmul(
                                psum_t[:, js], w_sb[:, k, ms], xb[:, k, js],
                                start=(k == 0), stop=(k == kt - 1),
                            )
                else:
                    for k in range(kt):
                        for j in range(nmm):
                            js = slice(j * 512, (j + 1) * 512)
                            nc.tensor.matmul(
                                psum_t[:, js], w_sb[:, k, ms], xb[:, k, js],
                                start=(k == 0), stop=(k == kt - 1),
                            )

            def emit_body():
                # per chan-group scan-state chain: AP of [128, 1]
                h_prev = [aux_sb[:, 0, m : m + 1] for m in range(mg)]
                nmm = nb // 512  # MMs per accumulation row-chunk (PSUM bank = 512 fp32)
                for blk in range(nblk):
                    xb = xpool.tile([128, kt, nb], dt, tag="xb", name="xb")
                    for k in range(kt):
                        nc.sync.dma_start(
                            xb[:, k, :],
                            xT_r[:, k, blk * nb : (blk + 1) * nb],
                        )

                    for m in range(mg):
                        ms = slice(m * 128, (m + 1) * 128)
                        kz = psum.tile([128, nb], F32, tag="kz", name="kz")
                        kh = psum.tile([128, nb], F32, tag="kh", name="kh")
                        emit_mms(kz, wz_sb, xb, ms, nmm)
                        emit_mms(kh, wh_sb, xb, ms, nmm)

                        a_t = epool.tile([128, nb], F32, tag="a", name="a_t")
                        z_t = epool.tile([128, nb], F32, tag="z", name="z_t")
                        sp_t = epool.tile([128, nb], F32, tag="sp", name="sp_t")
                        gp_t = epool.tile([128, nb], F32, tag="gp", name="gp_t")
                        b_t = epool.tile([128, nb], F32, tag="b", name="b_t")
                        h_t = hpool.tile([128, nb], F32, tag=f"h{m}", bufs=h_bufs, name="h_t")

                        # a = sigmoid(-(kz + bz));  z = sigmoid(kz + bz)
                        nc.scalar.activation(
                            a_t[:], kz[:], AF.Sigmoid,
                            bias=aux_sb[:, 2, m : m + 1], scale=-1.0,
                        )
                        nc.scalar.activation(
                            z_t[:], kz[:], AF.Sigmoid,
                            bias=aux_sb[:, 1, m : m + 1], scale=1.0,
                        )
                        # sp = sigmoid(kh + bh)
                        nc.scalar.activation(
                            sp_t[:], kh[:], AF.Sigmoid,
                            bias=aux_sb[:, 3, m : m + 1], scale=1.0,
                        )
                        # gp = max(kh + (bh+0.5), sp)
                        nc.vector.scalar_tensor_tensor(
                            gp_t[:], kh[:], aux_sb[:, 4, m : m + 1], sp_t[:],
                            op0=ALU.add, op1=ALU.max,
                        )
                        # b = z * gp
                        nc.vector.tensor_mul(b_t[:], z_t[:], gp_t[:])
                        # h scan: state = a*state + b
                        nc.vector.tensor_tensor_scan(
                            h_t[:], a_t[:], b_t[:], h_prev[m],
                            op0=ALU.mult, op1=ALU.add,
                        )
                        h_prev[m] = h_t[:, nb - 1 : nb]

                        nc.sync.dma_start(
                            hT_d.ap()[ms, blk * nb : (blk + 1) * nb], h_t[:]
                        )

            if loop_reps == 1:
                emit_body()
            else:
                with tc.For_i(0, loop_reps, 1):
                    emit_body()

    nc.compile()
    return nc


def _g(x):
    return np.where(x >= 0, x + 0.5, 1.0 / (1.0 + np.exp(-x)))


def make_in_maps(x, h_0, Wz, bz, Wh, bh, seq=SEQ, din=DIN, ch=CH, dt_in="f32r"):
    """Host-side shard: returns one in_map per core."""
    np_dt = mybir.dt.np(IN_DT[dt_in])
    mg = ch // 128
    gh0 = _g(h_0.astype(np.float32))  # [bs, 1, H]
    in_maps = []
    for c in range(NCORES):
        b, g = divmod(c, H_SPLIT)
        cs = slice(g * ch, (g + 1) * ch)
        aux = np.zeros((128, 5, mg), dtype=np.float32)
        aux[:, 0, :] = gh0[b, 0, cs].reshape(mg, 128).T
        aux[:, 1, :] = bz[cs].reshape(mg, 128).T
        aux[:, 2, :] = -bz[cs].reshape(mg, 128).T
        aux[:, 3, :] = bh[cs].reshape(mg, 128).T
        aux[:, 4, :] = (bh[cs] + 0.5).reshape(mg, 128).T
        in_maps.append(
            {
                "xT": np.ascontiguousarray(x[b].T.astype(np_dt)),
                "wzT": np.ascontiguousarray(Wz[cs, :].T.astype(np_dt)),
                "whT": np.ascontiguousarray(Wh[cs, :].T.astype(np_dt)),
                "aux": aux,
            }
        )
    return in_maps


_NC_CACHE = {}


def get_nc():
    if "nc" not in _NC_CACHE:
        _NC_CACHE["nc"] = build_nc()
    return _NC_CACHE["nc"]


def kernel(x, h_0, Wz, bz, Wh, bh, trace=False, trace_kwargs=None):
    x = np.asarray(x)
    h_0 = np.asarray(h_0)
    Wz = np.asarray(Wz)
    bz = np.asarray(bz)
    Wh = np.asarray(Wh)
    bh = np.asarray(bh)

    nc = get_nc()
    in_maps = make_in_maps(x, h_0, Wz, bz, Wh, bh)
    res = run_bass_kernel_spmd(
        nc, in_maps, core_ids=list(range(NCORES)),
        trace=trace, **(trace_kwargs or {}),
    )
    out = np.empty((BS, SEQ, H), dtype=np.float32)
    for c in range(NCORES):
        b, g = divmod(c, H_SPLIT)
        out[b, :, g * CH : (g + 1) * CH] = res.results[c]["hT"].T
    if trace:
        kernel.last_result = res
    return out


# revision 29
# speedup vs baseline: 1.8461x; 1.8461x over previous
"""MiniGRU Trainium2 kernel.

Problem: h_t = (1-z_t) h_{t-1} + z_t g(p_t), with
  z_t = sigmoid(x_t @ Wz^T + bz), p_t = x_t @ Wh^T + bh,
  g(x) = x + 0.5 for x>=0 else sigmoid(x)  (note g(x) = max(x+0.5, sigmoid(x))),
  initial state g(h_0).  Shapes: x [4, 4096, 1024], H = 1024.

Sharding: 8 cores = batch(4) x H-halves(2). No collectives. Each core gets
host-pre-transposed inputs:
  xT  [1024 din, 4096 seq]   (moving operand for both GEMMs)
  wzT/whT [1024 din, 512 ch] (stationary operands)
  aux [128, 5, 4]            per chan-group columns: +-g(h0), bz, -bz, bh, bh+0.5
and returns hT [512 ch, 4096 seq]; host transposes back.

Device dataflow per (seq-block of nb=512, chan-group of 128):
  PE: 8+8 accumulating matmuls -> PSUM kz, kh [128 ch, 512 seq]; the kz and
      kh chains are interleaved (pairi) so consecutive matmuls write
      different PSUM banks and kz closes early for the ACT consumer.
  ACT: a = sigmoid(-kz-bz), sp = sigmoid(kh+bh)
  DVE: gp = max(kh+(bh+0.5), sp); b' = (a-1)*gp = -z*gp;
       s = tensor_tensor_scan(a, b', init)  -- state = a*state + b' along seq
With init = -g(h0) the scan tracks s = -h exactly (linearity), which removes
the z=sigmoid(kz+bz) activation entirely; the host negates the output.
Scan state chains across seq-blocks via initial=prev_s[:, -1:].

The GEMM inputs go to the PE as bfloat16 (x and W quantized on host, rel err
~1.3e-3): 16-bit streams at the same 1 column/cycle as fp32r, halves the x
DMA traffic, and gets the auto-emitted LDWEIGHTS prefetch pipeline. The
output is stored as fp16 (halves DMA-out; scan state itself only crosses
fp16 at the 8 block boundaries). PSUM accumulation and all elementwise math
are fp32.

Measured (paired For_i-slope, 8-core axon trn2): ~138 us/exec sustained vs
~150 us for the fp32r baseline; PE-bound (matmul-only ablation measures the
same ~139 us, so overlap of ACT/DVE/scan/DMA behind the PE is ~complete).
"""

import numpy as np

import concourse.bass as bass
import concourse.bacc as bacc
import concourse.mybir as mybir
import concourse.tile as tile
from concourse.bass_utils import run_bass_kernel_spmd

F32 = mybir.dt.float32
F32R = mybir.dt.float32r
F16 = mybir.dt.float16
BF16 = mybir.dt.bfloat16
AF = mybir.ActivationFunctionType
ALU = mybir.AluOpType

BS, SEQ, DIN, H = 4, 4096, 1024, 1024
NCORES = 8
H_SPLIT = 2
CH = H // H_SPLIT  # channels per core

IN_DT = {"f32r": F32R, "f16": F16, "bf16": BF16}


def build_nc(seq=SEQ, din=DIN, ch=CH, nb=512, x_bufs=4, loop_reps=1,
             epool_bufs=3, h_bufs=4, psum_bufs=None, dt_in="f32r", korder="j",
             out_dt="f32", ldw=False, out_ring="sync", body_reps=1, mmn=512,
             kt_mm=None, consumers=True, scan=True, pairi=False, negh=False,
             b_eng="vector", samew=False):
    """Build the single-core SPMD Bass program.

    loop_reps > 1 wraps the whole body in a hardware For_i loop that
    recomputes the same output N times — used only for benchmarking
    (slope of wall time vs reps isolates HW exec time from RPC overhead).
    body_reps emits the body multiple times inside one iteration (to
    amortize the For_i back-edge when measuring).

    korder="k" orders the accumulation loops stationary-major (all PSUM
    row-chunks for one weight tile before moving to the next), reusing the
    loaded stationary across nb//512 matmuls.

    ldw=True emits an explicit ldweights before each matmul (16-bit dtypes
    only) so the PE can prefetch the stationary into the background weight
    buffer while the previous matmul streams.
    """
    kt = din // 128   # contraction tiles
    if kt_mm is None:
        kt_mm = kt    # emit fewer k-tiles than kt for PE-scaling ablations
    mg = ch // 128    # chan groups
    nblk = seq // nb  # seq blocks
    if psum_bufs is None:
        psum_bufs = max(1, 8 // (2 * (nb // 512)))  # kz+kh tags fill all 8 banks

    dt = IN_DT[dt_in]
    odt = F32 if out_dt == "f32" else F16
    nc = bacc.Bacc("TRN2", target_bir_lowering=False, debug=False)

    xT_d = nc.dram_tensor("xT", [din, seq], dt, kind="ExternalInput")
    wzT_d = nc.dram_tensor("wzT", [din, ch], dt, kind="ExternalInput")
    whT_d = nc.dram_tensor("whT", [din, ch], dt, kind="ExternalInput")
    aux_d = nc.dram_tensor("aux", [128, 5, mg], F32, kind="ExternalInput")
    hT_d = nc.dram_tensor("hT", [ch, seq], odt, kind="ExternalOutput")

    xT_r = xT_d.ap().rearrange("(k p) s -> p k s", p=128)
    wzT_r = wzT_d.ap().rearrange("(k p) c -> p k c", p=128)
    whT_r = whT_d.ap().rearrange("(k p) c -> p k c", p=128)

    with tile.TileContext(nc) as tc:
        with (
            tc.tile_pool(name="wpool", bufs=1) as wpool,
            tc.tile_pool(name="xpool", bufs=x_bufs) as xpool,
            tc.tile_pool(name="epool", bufs=epool_bufs) as epool,
            tc.tile_pool(name="hpool", bufs=1) as hpool,
            tc.tile_pool(name="psum", bufs=psum_bufs, space="PSUM") as psum,
        ):
            wz_sb = wpool.tile([128, kt, ch], dt)
            wh_sb = wpool.tile([128, kt, ch], dt)
            aux_sb = wpool.tile([128, 5, mg], F32)
            nc.sync.dma_start(aux_sb[:], aux_d.ap())
            # per-k W loads so the first matmul waits only on its own slice;
            # issued on the scalar HWDGE ring so they don't queue ahead of the
            # first x-block loads on the sync ring.
            for k in range(kt):
                nc.scalar.dma_start(wz_sb[:, k, :], wzT_r[:, k, :])
                nc.scalar.dma_start(wh_sb[:, k, :], whT_r[:, k, :])

            def mm_one(psum_t, w_sb, xb, ms, j, k):
                js = slice(j * mmn, (j + 1) * mmn)
                kw_ = 0 if samew else k  # samew: timing probe, wrong math
                if ldw:
                    nc.tensor.ldweights(w_sb[:, kw_, ms])
                nc.tensor.matmul(
                    psum_t[:, js], w_sb[:, kw_, ms], xb[:, k, js],
                    start=(k == 0), stop=(k == kt_mm - 1),
                )

            def emit_mms(psum_t, w_sb, xb, ms, nmm):
                if korder == "j":
                    for j in range(nmm):
                        for k in range(kt_mm):
                            mm_one(psum_t, w_sb, xb, ms, j, k)
                else:
                    for k in range(kt_mm):
                        for j in range(nmm):
                            mm_one(psum_t, w_sb, xb, ms, j, k)

            def emit_mms_paired(kz, kh, wz_sb, wh_sb, xb, ms, nmm):
                # alternate PSUM target banks so back-to-back matmuls never
                # accumulate into the same bank region
                for k in range(kt_mm):
                    for j in range(nmm):
                        mm_one(kz, wz_sb, xb, ms, j, k)
                        mm_one(kh, wh_sb, xb, ms, j, k)

            def emit_body():
                # per chan-group scan-state chain: AP of [128, 1]
                h_prev = [aux_sb[:, 0, m : m + 1] for m in range(mg)]
                nmm = nb // mmn  # MMs per accumulation row-chunk
                for blk in range(nblk):
                    xb = xpool.tile([128, kt, nb], dt, tag="xb", name="xb")
                    for k in range(kt):
                        nc.sync.dma_start(
                            xb[:, k, :],
                            xT_r[:, k, blk * nb : (blk + 1) * nb],
                        )

                    for m in range(mg):
                        ms = slice(m * 128, (m + 1) * 128)
                        kz = psum.tile([128, nb], F32, tag="kz", name="kz")
                        kh = psum.tile([128, nb], F32, tag="kh", name="kh")
                        if pairi:
                            emit_mms_paired(kz, kh, wz_sb, wh_sb, xb, ms, nmm)
                        else:
                            emit_mms(kz, wz_sb, xb, ms, nmm)
                            emit_mms(kh, wh_sb, xb, ms, nmm)

                        if not consumers:
                            continue

                        a_t = epool.tile([128, nb], F32, tag="a", name="a_t")
                        sp_t = epool.tile([128, nb], F32, tag="sp", name="sp_t")
                        gp_t = epool.tile([128, nb], F32, tag="gp", name="gp_t")
                        b_t = epool.tile([128, nb], F32, tag="b", name="b_t")
                        h_t = hpool.tile([128, nb], odt, tag=f"h{m}", bufs=h_bufs, name="h_t")

                        # a = sigmoid(-(kz + bz))
                        nc.scalar.activation(
                            a_t[:], kz[:], AF.Sigmoid,
                            bias=aux_sb[:, 2, m : m + 1], scale=-1.0,
                        )
                        # sp = sigmoid(kh + bh)
                        nc.scalar.activation(
                            sp_t[:], kh[:], AF.Sigmoid,
                            bias=aux_sb[:, 3, m : m + 1], scale=1.0,
                        )
                        # gp = max(kh + (bh+0.5), sp)
                        nc.vector.scalar_tensor_tensor(
                            gp_t[:], kh[:], aux_sb[:, 4, m : m + 1], sp_t[:],
                            op0=ALU.add, op1=ALU.max,
                        )
                        if negh:
                            # b' = (a-1)*gp = -z*gp; with init -g(h0) the scan
                            # then tracks s = -h (host negates the output)
                            b_engine = nc.vector if b_eng == "vector" else nc.gpsimd
                            b_engine.scalar_tensor_tensor(
                                b_t[:], a_t[:], 1.0, gp_t[:],
                                op0=ALU.subtract, op1=ALU.mult,
                            )
                        else:
                            # z = sigmoid(kz + bz);  b = z * gp
                            z_t = epool.tile([128, nb], F32, tag="z", name="z_t")
                            nc.scalar.activation(
                                z_t[:], kz[:], AF.Sigmoid,
                                bias=aux_sb[:, 1, m : m + 1], scale=1.0,
                            )
                            nc.vector.tensor_mul(b_t[:], z_t[:], gp_t[:])
                        # h scan: state = a*state + b
                        if scan:
                            nc.vector.tensor_tensor_scan(
                                h_t[:], a_t[:], b_t[:], h_prev[m],
                                op0=ALU.mult, op1=ALU.add,
                            )
                            h_prev[m] = h_t[:, nb - 1 : nb]
                        else:  # timing ablation only — wrong math
                            nc.vector.tensor_mul(h_t[:], a_t[:], b_t[:])

                        out_eng = nc.sync if out_ring == "sync" else nc.scalar
                        out_eng.dma_start(
                            hT_d.ap()[ms, blk * nb : (blk + 1) * nb], h_t[:]
                        )

            def emit_bodies():
                for _ in range(body_reps):
                    emit_body()

            if loop_reps == 1:
                emit_bodies()
            else:
                with tc.For_i(0, loop_reps, 1):
                    emit_bodies()

    nc.compile()
    return nc


def _g(x):
    return np.where(x >= 0, x + 0.5, 1.0 / (1.0 + np.exp(-x)))


def make_in_maps(x, h_0, Wz, bz, Wh, bh, seq=SEQ, din=DIN, ch=CH, dt_in="f32r",
                 negh=False):
    """Host-side shard: returns one in_map per core."""
    np_dt = mybir.dt.np(IN_DT[dt_in])
    mg = ch // 128
    gh0 = _g(h_0.astype(np.float32))  # [bs, 1, H]
    if negh:
        gh0 = -gh0
    in_maps = []
    for c in range(NCORES):
        b, g = divmod(c, H_SPLIT)
        cs = slice(g * ch, (g + 1) * ch)
        aux = np.zeros((128, 5, mg), dtype=np.float32)
        aux[:, 0, :] = gh0[b, 0, cs].reshape(mg, 128).T
        aux[:, 1, :] = bz[cs].reshape(mg, 128).T
        aux[:, 2, :] = -bz[cs].reshape(mg, 128).T
        aux[:, 3, :] = bh[cs].reshape(mg, 128).T
        aux[:, 4, :] = (bh[cs] + 0.5).reshape(mg, 128).T
        in_maps.append(
            {
                "xT": np.ascontiguousarray(x[b].T.astype(np_dt)),
                "wzT": np.ascontiguousarray(Wz[cs, :].T.astype(np_dt)),
                "whT": np.ascontiguousarray(Wh[cs, :].T.astype(np_dt)),
                "aux": aux,
            }
        )
    return in_maps


# Shipped configuration: bf16 GEMM inputs (rel err ~1.3e-3 vs fp32 reference;
# measured a few us faster sustained than fp16, consistent with lower
# multiplier switching power easing the PE throttle), fp16 output, scan
# tracking s = -h (z activation folded away via b' = (a-1)*gp; host negates),
# kz/kh matmul chains interleaved across PSUM banks.
BEST_KW = dict(dt_in="bf16", negh=True, out_dt="f16", pairi=True)

_NC_CACHE = {}


def get_nc():
    if "nc" not in _NC_CACHE:
        _NC_CACHE["nc"] = build_nc(**BEST_KW)
    return _NC_CACHE["nc"]


def kernel(x, h_0, Wz, bz, Wh, bh, trace=False, trace_kwargs=None):
    x = np.asarray(x)
    h_0 = np.asarray(h_0)
    Wz = np.asarray(Wz)
    bz = np.asarray(bz)
    Wh = np.asarray(Wh)
    bh = np.asarray(bh)

    nc = get_nc()
    in_maps = make_in_maps(
        x, h_0, Wz, bz, Wh, bh,
        dt_in=BEST_KW.get("dt_in", "f32r"), negh=BEST_KW.get("negh", False),
    )
    res = run_bass_kernel_spmd(
        nc, in_maps, core_ids=list(range(NCORES)),
        trace=trace, **(trace_kwargs or {}),
    )
    sign = -1.0 if BEST_KW.get("negh") else 1.0
    out = np.empty((BS, SEQ, H), dtype=np.float32)
    for c in range(NCORES):
        b, g = divmod(c, H_SPLIT)
        out[b, :, g * CH : (g + 1) * CH] = sign * res.results[c]["hT"].T.astype(
            np.float32
        )
    if trace:
        kernel.last_result = res
    return out
